# revision 56
# baseline (speedup 1.0000x reference)
"""Causal multi-head attention (B=16, T=1024, E=1024, H=16, Dh=64) on 8 TRN2
NeuronCores.

Sharding: data-parallel over batch -- 2 batch elements per core, weights
replicated, no collectives. Host pre-transposes x and packs weights; each core
runs an identical Bass/Tile program on its shard.

Per-core dataflow (all in "transposed" orientation so no on-chip transposes
are ever needed):
  x^T [E,T] (host)   --matmul-->  Q^T,K^T [Dh,T] per head (head-pairs packed
                                  into 128 partitions; 1/sqrt(Dh) folded into
                                  the Q PSUM->SBUF copy)
                     --matmul-->  V [T,Dh] per head (+ ones column)
  S^T[tk,tq] = (K^T tile).T @ Q^T  per key-tile, causal tiles skipped
  P^T = exp(S^T) on ScalarE (scores are O(1): no max subtraction needed);
        diagonal tiles masked by 0/1 multiply
  O'^T[65,tq] += (V'|1).T @ P^T   -- row 64 accumulates the softmax denom
  Y^T = O'^T[0:64] * bcast(1/denom)
  out[t,E] = Y^T.T @ Wo + bo

Scheduling notes (hard-won on HW):
  - engine APs need partition base in {0,32,64,96}; partition_broadcast reads
    physical partition 0 regardless of the AP; tensor_tensor wants equal
    bases when both operands are SBUF.
  - HAM unthrottles the PE (1.2 -> 2.4 GHz) only on a fully-busy 3.4us
    window, so each pair's Q/K projection matmuls are interleaved into the
    previous pair's attention stream as dense filler.
  - normalization (reciprocal + broadcast + scale) is drip-emitted between
    i-steps so DVE/GpSimd bursts never starve the PE's PV chain.
"""
import numpy as np
import ml_dtypes

import concourse.bass as bass
import concourse.mybir as mybir
import concourse.tile as tile
from concourse import bacc
from concourse.bass_utils import run_bass_kernel_spmd

B, T, E = 16, 1024, 1024
H, Dh = 16, 64
NCORES = 8
BL = B // NCORES          # batches per core
P = 128                   # partitions
ET = E // P               # 8 tiles along E / token / hd dims
HP = H // 2               # 8 head-pairs
BF = mybir.dt.bfloat16
F32 = mybir.dt.float32
AF = mybir.ActivationFunctionType

_CACHE = {}


def _pieces(i):
    """Column pieces of [128*i, 1024) that do not cross the 512 PSUM-bank
    boundary."""
    if i < 4:
        return [(128 * i, 512), (512, 1024)]
    return [(128 * i, 1024)]


def _build(dbg=False):
    nc = bacc.Bacc("TRN2", target_bir_lowering=False, debug=False,
                   num_devices=NCORES)

    dbg_out = {}
    if dbg:
        for name, shape, dt in [
            ("d_qT", [P, HP, T], BF), ("d_kT", [P, HP, T], BF),
            ("d_v", [P, ET, H, Dh + 1], BF), ("d_pt", [ET, P, T], BF),
            ("d_op", [P, T], F32),
            ("d_r1", [1, T], F32), ("d_rb", [Dh, T], F32),
            ("d_yT", [P, HP, T], BF),
        ]:
            dbg_out[name] = nc.dram_tensor(name, shape, dt,
                                           kind="ExternalOutput").ap()

    xT = nc.dram_tensor("xT", [BL, E, T], BF, kind="ExternalInput").ap()
    wq = nc.dram_tensor("wq", [E, H * Dh], BF, kind="ExternalInput").ap()
    wk = nc.dram_tensor("wk", [E, H * Dh], BF, kind="ExternalInput").ap()
    wv = nc.dram_tensor("wv", [E, H * Dh], BF, kind="ExternalInput").ap()
    wo = nc.dram_tensor("wo", [H * Dh, E], BF, kind="ExternalInput").ap()
    borep = nc.dram_tensor("borep", [P, E], F32, kind="ExternalInput").ap()
    mask01 = nc.dram_tensor("mask01", [P, P], BF, kind="ExternalInput").ap()
    out = nc.dram_tensor("out", [BL, T, E], F32, kind="ExternalOutput").ap()

    with tile.TileContext(nc) as tc:
        with (
            tc.tile_pool(name="consts", bufs=1) as cpool,
            tc.tile_pool(name="xp", bufs=1) as xpool,
            tc.tile_pool(name="qk", bufs=1) as qkpool,
            tc.tile_pool(name="vp2", bufs=2) as vpool,
            tc.tile_pool(name="vy", bufs=1) as vypool,
            tc.tile_pool(name="pt", bufs=6) as ptpool,
            tc.tile_pool(name="sm", bufs=2) as spool,
            tc.tile_pool(name="dn", bufs=1) as dnpool,
            tc.tile_pool(name="ob", bufs=3) as opool,
            tc.tile_pool(name="pso", bufs=3, space="PSUM") as pso,
            tc.tile_pool(name="psc", bufs=2, space="PSUM") as psc,
        ):
            # DMA order matters for the startup critical path: V-projection
            # only needs Wv + xT, so those go first; Wo/bias are not needed
            # until the output projection
            wq_sb = cpool.tile([P, ET, H * Dh], BF, tag="wq")
            wk_sb = cpool.tile([P, ET, H * Dh], BF, tag="wk")
            wv_sb = cpool.tile([P, ET, H * Dh], BF, tag="wv")
            wo_sb = cpool.tile([P, ET, E], BF, tag="wo")
            nc.sync.dma_start(wv_sb[:], wv.rearrange("(n p) c -> p n c", p=P))

            xT_tiles = {}
            v_tiles = {}

            def load_blocks(b):
                """xT load + V-projection for batch b as dense PE filler
                blocks (also usable as pending entries during the previous
                batch's last attention pairs)."""
                def ld(b=b):
                    xT_tiles[b] = xpool.tile([P, ET, T], BF, tag="xT",
                                             name=f"xT{b}")
                    nc.sync.dma_start(
                        xT_tiles[b][:],
                        xT[b].rearrange("(n p) c -> p n c", p=P))
                    v_tiles[b] = vpool.tile([P, ET, H, Dh + 1], BF, tag="v",
                                            name=f"v{b}")
                    nc.vector.memset(v_tiles[b][:, :, :, Dh], 1.0)
                blocks = [ld]
                for t in range(ET):
                    for n2 in range(2):
                        def vblk(t=t, n2=n2, b=b):
                            cs = slice(512 * n2, 512 * (n2 + 1))
                            vp = psc.tile([P, 512], F32, tag="pc",
                                          name=f"vp{b}_{t}_{n2}")
                            for i in range(ET):
                                nc.tensor.matmul(
                                    vp[:],
                                    lhsT=xT_tiles[b][:, i,
                                                     128 * t:128 * (t + 1)],
                                    rhs=wv_sb[:, i, cs],
                                    start=(i == 0), stop=(i == ET - 1),
                                )
                            nc.scalar.activation(
                                v_tiles[b][:, t, 8 * n2:8 * (n2 + 1), 0:Dh],
                                vp[:].rearrange("p (h d) -> p h d", d=Dh),
                                AF.Copy,
                            )
                        blocks.append(vblk)
                return blocks

            blocks0 = load_blocks(0)
            blocks0[0]()  # xT load right behind Wv
            nc.sync.dma_start(wq_sb[:], wq.rearrange("(n p) c -> p n c", p=P))
            nc.sync.dma_start(wk_sb[:], wk.rearrange("(n p) c -> p n c", p=P))
            mask_sb = cpool.tile([P, P], BF, tag="mask")
            nc.sync.dma_start(mask_sb[:], mask01)
            for blk in blocks0[1:]:
                blk()
            nc.sync.dma_start(wo_sb[:], wo.rearrange("(n p) c -> p n c", p=P))
            borep_sb = cpool.tile([P, E], F32, tag="bo")
            nc.sync.dma_start(borep_sb[:], borep)

            pending = []

            def drain(n):
                for _ in range(min(n, len(pending))):
                    pending.pop(0)()

            for b in range(BL):
                xT_sb = xT_tiles[b]
                v_sb = v_tiles[b]

                # ---- Q^T / K^T projections, emitted as closures so pair
                # pp's projection interleaves into pair pp-1's attention ----
                qT = qkpool.tile([P, HP, T], BF, tag="q", name=f"q{b}")
                kT = qkpool.tile([P, HP, T], BF, tag="k", name=f"k{b}")

                def proj_subblocks(pp, b=b, qT=qT, kT=kT, xT_sb=xT_sb):
                    blocks = []
                    for (lbl, w_sb, dst, scale) in (("q", wq_sb, qT, 0.125),
                                                    ("k", wk_sb, kT, 1.0)):
                        for n2 in range(2):
                            def blk(lbl=lbl, w_sb=w_sb, dst=dst, scale=scale,
                                    n2=n2, pp=pp, b=b):
                                cs = slice(512 * n2, 512 * (n2 + 1))
                                pj = psc.tile(
                                    [P, 512], F32, tag="pc",
                                    name=f"pj{b}_{pp}_{n2}_{lbl}")
                                for i in range(ET):
                                    nc.tensor.matmul(
                                        pj[:],
                                        lhsT=w_sb[:, i,
                                                  128 * pp:128 * (pp + 1)],
                                        rhs=xT_sb[:, i, cs],
                                        start=(i == 0), stop=(i == ET - 1),
                                    )
                                nc.scalar.activation(dst[:, pp, cs], pj[:],
                                                     AF.Copy, scale=scale)
                            blocks.append(blk)
                    return blocks

                for blk in proj_subblocks(0):
                    blk()

                # ---- attention: pairs of heads, drip-scheduled extras ----
                yT = vypool.tile([P, HP, T], BF, tag="y", name=f"y{b}")

                def yT_ap(hp):
                    return yT[:, hp, :]
                den = dnpool.tile([P, 4, T], F32, tag="den")
                nc.vector.memset(den[:], 1.0)

                def queue_normalize(g, b=b, yT_ap=yT_ap, den=den,
                                    half=None):
                    # half=0/1 reciprocates only partitions [0:64)/[64:128)
                    # of the slot (heads 4g..4g+1 / 4g+2..4g+3) so the last
                    # quad can normalize pair-by-pair instead of in one
                    # end-of-batch burst
                    p0, p1 = (0, P) if half is None else \
                        (64 * half, 64 * half + 64)
                    for c in range(8):
                        def recip_chunk(g=g, c=c, p0=p0, p1=p1):
                            nc.vector.reciprocal(
                                den[p0:p1, g, 128 * c:128 * (c + 1)],
                                den[p0:p1, g, 128 * c:128 * (c + 1)])
                        pending.append(recip_chunk)
                    if dbg and b == 0 and g == 0 and half is None:
                        pending.append(lambda: nc.sync.dma_start(
                            dbg_out["d_r1"], den[0:1, 0, :]))
                    heads = range(4 * g, 4 * g + 4) if half is None else \
                        range(4 * g + 2 * half, 4 * g + 2 * half + 2)
                    for h in heads:
                        holder = {}

                        def stage(h=h, b=b, holder=None):
                            hp, po = h // 2, Dh * (h % 2)
                            pb = 32 * (h % 4)
                            r1 = spool.tile([1, T], BF, tag="r1",
                                            name=f"r1_{b}_{h}")
                            nc.scalar.activation(
                                r1[:], den[pb:pb + 1, h // 4, :], AF.Copy)
                            rb = spool.tile([P, T], BF, tag="rb",
                                            name=f"rb_{b}_{h}")
                            nc.gpsimd.partition_broadcast(rb[:], r1[:])
                            if dbg and b == 0 and h == 1:
                                nc.sync.dma_start(dbg_out["d_rb"],
                                                  rb[0:Dh, :])
                            holder['rb'] = rb

                        def mul_step(h=h, holder=holder):
                            hp, po = h // 2, Dh * (h % 2)
                            ap = yT_ap(hp)
                            nc.vector.tensor_mul(
                                ap[po:po + Dh, :], ap[po:po + Dh, :],
                                holder['rb'][po:po + Dh, :])

                        pending.append(
                            lambda h=h, holder=holder: stage(h, b, holder))
                        pending.append(mul_step)

                for hp in range(HP):
                    if hp + 1 < HP:
                        # front of the queue: pair hp+1's projection must
                        # finish within this pair's attention
                        pending[0:0] = proj_subblocks(hp + 1)
                    if hp == 6 and b + 1 < BL:
                        # next batch's x load + V projection: dense PE
                        # filler for the last two pairs (which have no
                        # projection blocks of their own)
                        pending.extend(load_blocks(b + 1))
                    ops = [pso.tile([P, 1024], F32, tag="op",
                                    name=f"op{b}_{hp}_{s}") for s in range(2)]
                    for i in range(ET):
                        pts = []
                        for sub in (0, 1):
                            po = Dh * sub
                            pt = ptpool.tile([P, 1024], BF, tag="pt",
                                             name=f"pt{b}_{hp}_{i}_{sub}")
                            for (a0, a1) in _pieces(i):
                                sp_ = psc.tile([P, 512], F32, tag="pc",
                                               name=f"sp{b}_{hp}_{i}_{sub}_{a0}")
                                w = a1 - a0
                                nc.tensor.matmul(
                                    sp_[:, 0:w],
                                    lhsT=kT[po:po + Dh, hp,
                                            128 * i:128 * (i + 1)],
                                    rhs=qT[po:po + Dh, hp, a0:a1],
                                    start=True, stop=True,
                                )
                                nc.scalar.activation(pt[:, a0:a1],
                                                     sp_[:, 0:w], AF.Exp)
                            ds_ = slice(128 * i, 128 * (i + 1))
                            nc.vector.tensor_mul(pt[:, ds_], pt[:, ds_],
                                                 mask_sb[:])
                            pts.append(pt)
                            if dbg and b == 0 and hp == 0 and sub == 0:
                                nc.sync.dma_start(dbg_out["d_pt"][i], pt[:])
                        for sub in (0, 1):
                            h = 2 * hp + sub
                            for (a0, a1) in _pieces(i):
                                nc.tensor.matmul(
                                    ops[sub][0:Dh + 1, a0:a1],
                                    lhsT=v_sb[:, i, h, :],
                                    rhs=pts[sub][:, a0:a1],
                                    start=(i == 0), stop=(i == ET - 1),
                                    skip_group_check=True,
                                )
                            if i == ET - 1:
                                po = Dh * sub
                                nc.scalar.activation(
                                    yT_ap(hp)[po:po + Dh, :],
                                    ops[sub][0:Dh, :], AF.Copy)
                                pb = 32 * (h % 4)
                                nc.vector.tensor_copy(
                                    den[pb:pb + 1, h // 4, :],
                                    ops[sub][Dh:Dh + 1, :])
                                if dbg and b == 0 and h == 0:
                                    opc = dnpool.tile([P, T], F32,
                                                      tag="dbg_op")
                                    nc.vector.tensor_copy(opc[:],
                                                          ops[sub][:])
                                    nc.sync.dma_start(dbg_out["d_op"],
                                                      opc[:])
                            drain(1)
                    if hp == HP - 2:
                        queue_normalize(3, half=0)
                    elif hp == HP - 1:
                        queue_normalize(3, half=1)
                    elif hp % 2 == 1:
                        queue_normalize(hp // 2)

                # emission order IS dependency order under Tile's tracer:
                # all normalize muls must be emitted before out-proj reads yT
                drain(len(pending))
                if dbg and b == 0:
                    nc.sync.dma_start(dbg_out["d_qT"], qT[:])
                    nc.sync.dma_start(dbg_out["d_kT"], kT[:])
                    nc.sync.dma_start(dbg_out["d_v"], v_sb[:])
                    nc.sync.dma_start(dbg_out["d_yT"], yT[:])

                # ---- output projection + bias ----
                # for non-final batches the half-blocks are deferred into the
                # NEXT batch's attention stream (dense PE filler that
                # overlaps what used to be a serial phase)
                def outproj_blocks(b=b, yT=yT):
                    blocks = []
                    for t in range(ET):
                        for n2 in range(2):
                            def oblk(t=t, n2=n2, b=b, yT=yT):
                                cs = slice(512 * n2, 512 * (n2 + 1))
                                o2 = psc.tile([P, 512], F32, tag="pc",
                                              name=f"o2_{b}_{t}_{n2}")
                                for j in range(ET):
                                    nc.tensor.matmul(
                                        o2[:],
                                        lhsT=yT[:, j,
                                                128 * t:128 * (t + 1)],
                                        rhs=wo_sb[:, j, cs],
                                        start=(j == 0), stop=(j == ET - 1),
                                    )
                                ob = opool.tile([P, 512], F32, tag="ob",
                                                name=f"ob{b}_{t}_{n2}")
                                nc.vector.tensor_add(ob[:], o2[:],
                                                     borep_sb[:, cs])
                                nc.sync.dma_start(
                                    out[b, 128 * t:128 * (t + 1), cs],
                                    ob[:])
                            blocks.append(oblk)
                    return blocks

                # b's normalize must be fully emitted before anything of
                # batch b+1 (den/yT pool slots are reused)
                drain(len(pending))
                blocks = outproj_blocks()
                if b + 1 < BL:
                    # defer half into the next batch's attention stream;
                    # they must all drain before b+1's first yT write
                    # (pair-0 end), which bounds the deferral to ~12 entries
                    for blk in blocks[:8]:
                        blk()
                    pending.extend(blocks[8:])
                else:
                    for blk in blocks:
                        blk()
            drain(len(pending))

    nc.compile()
    return nc


def _get_nc():
    if "nc" not in _CACHE:
        _CACHE["nc"] = _build()
    return _CACHE["nc"]


def _prep_in_maps(x, Wq, Wk, Wv, Wo, bo):
    bf16 = ml_dtypes.bfloat16
    # [B,T,E] -> [B,E,T] transposed activations
    xT = np.ascontiguousarray(np.asarray(x).transpose(0, 2, 1)).astype(bf16)
    # [H,E,Dh] -> [E, H*Dh] (heads side by side so a 128-col slice = 2 heads)
    wq_pk = np.ascontiguousarray(
        np.asarray(Wq).transpose(1, 0, 2).reshape(E, H * Dh)).astype(bf16)
    wk_pk = np.ascontiguousarray(
        np.asarray(Wk).transpose(1, 0, 2).reshape(E, H * Dh)).astype(bf16)
    wv_pk = np.ascontiguousarray(
        np.asarray(Wv).transpose(1, 0, 2).reshape(E, H * Dh)).astype(bf16)
    wo_b = np.ascontiguousarray(np.asarray(Wo)).astype(bf16)
    borep = np.ascontiguousarray(
        np.broadcast_to(np.asarray(bo, np.float32), (P, E)))
    ii, jj = np.mgrid[0:P, 0:P]
    mask01 = (jj >= ii).astype(bf16)  # S^T[tk,tq]: keep tq >= tk

    in_maps = []
    for c in range(NCORES):
        in_maps.append({
            "xT": xT[BL * c:BL * (c + 1)],
            "wq": wq_pk, "wk": wk_pk, "wv": wv_pk, "wo": wo_b,
            "borep": borep, "mask01": mask01,
        })
    return in_maps


def run(inputs, trace=False):
    """Returns (full_output [B,T,E] fp32, BassKernelResults)."""
    nc = _get_nc()
    in_maps = _prep_in_maps(**inputs)
    res = run_bass_kernel_spmd(nc, in_maps, core_ids=list(range(NCORES)),
                               trace=trace)
    out = np.concatenate([res.results[c]["out"] for c in range(NCORES)],
                         axis=0)
    return out, res


def kernel(x, Wq, Wk, Wv, Wo, bo):
    out, _ = run(dict(x=x, Wq=Wq, Wk=Wk, Wv=Wv, Wo=Wo, bo=bo))
    return out


# revision 57
# speedup vs baseline: 1.2448x; 1.2448x over previous
"""Causal multi-head attention (B=16, T=1024, E=1024, H=16, Dh=64) on 8 TRN2
NeuronCores.

Sharding: data-parallel over batch -- 2 batch elements per core, weights
replicated, no collectives. Host pre-transposes x and packs weights; each core
runs an identical Bass/Tile program on its shard.

Per-core dataflow (all in "transposed" orientation so no on-chip transposes
are ever needed):
  x^T [E,T] (host)   --matmul-->  Q^T,K^T [Dh,T] per head (head-pairs packed
                                  into 128 partitions; 1/sqrt(Dh) folded into
                                  the Q PSUM->SBUF copy)
                     --matmul-->  V [T,Dh] per head (+ ones column)
  S^T[tk,tq] = (K^T tile).T @ Q^T  per key-tile, causal tiles skipped
  P^T = exp(S^T) on ScalarE (scores are O(1): no max subtraction needed);
        diagonal tiles masked by 0/1 multiply
  O'^T[65,tq] += (V'|1).T @ P^T   -- row 64 accumulates the softmax denom
  Y^T = O'^T[0:64] * bcast(1/denom)
  out[t,E] = Y^T.T @ Wo + bo

Scheduling notes (hard-won on HW):
  - engine APs need partition base in {0,32,64,96}; partition_broadcast reads
    physical partition 0 regardless of the AP; tensor_tensor wants equal
    bases when both operands are SBUF.
  - HAM unthrottles the PE (1.2 -> 2.4 GHz) only on a fully-busy 3.4us
    window, so each pair's Q/K projection matmuls are interleaved into the
    previous pair's attention stream as dense filler.
  - normalization (reciprocal + broadcast + scale) is drip-emitted between
    i-steps so DVE/GpSimd bursts never starve the PE's PV chain.
"""
import numpy as np
import ml_dtypes

import concourse.bass as bass
import concourse.mybir as mybir
import concourse.tile as tile
from concourse import bacc
from concourse.bass_utils import run_bass_kernel_spmd

B, T, E = 16, 1024, 1024
H, Dh = 16, 64
NCORES = 8
BL = B // NCORES          # batches per core
P = 128                   # partitions
ET = E // P               # 8 tiles along E / token / hd dims
HP = H // 2               # 8 head-pairs
BF = mybir.dt.bfloat16
F32 = mybir.dt.float32
AF = mybir.ActivationFunctionType

_CACHE = {}


def _pieces(i):
    """Column pieces of [128*i, 1024) that do not cross the 512 PSUM-bank
    boundary."""
    if i < 4:
        return [(128 * i, 512), (512, 1024)]
    return [(128 * i, 1024)]


def _build(dbg=False):
    nc = bacc.Bacc("TRN2", target_bir_lowering=False, debug=False,
                   num_devices=NCORES)

    dbg_out = {}
    if dbg:
        for name, shape, dt in [
            ("d_qT", [P, HP, T], BF), ("d_kT", [P, HP, T], BF),
            ("d_v", [P, ET, H, Dh + 1], BF), ("d_pt", [ET, P, T], BF),
            ("d_op", [P, T], F32),
            ("d_r1", [1, T], F32), ("d_rb", [Dh, T], F32),
            ("d_yT", [P, HP, T], BF),
        ]:
            dbg_out[name] = nc.dram_tensor(name, shape, dt,
                                           kind="ExternalOutput").ap()

    xT = nc.dram_tensor("xT", [BL, E, T], BF, kind="ExternalInput").ap()
    wq = nc.dram_tensor("wq", [E, H * Dh], BF, kind="ExternalInput").ap()
    wk = nc.dram_tensor("wk", [E, H * Dh], BF, kind="ExternalInput").ap()
    wv = nc.dram_tensor("wv", [E, H * Dh], BF, kind="ExternalInput").ap()
    wo = nc.dram_tensor("wo", [H * Dh, E], BF, kind="ExternalInput").ap()
    borep = nc.dram_tensor("borep", [P, E], F32, kind="ExternalInput").ap()
    mask01 = nc.dram_tensor("mask01", [P, P], BF, kind="ExternalInput").ap()
    out = nc.dram_tensor("out", [BL, T, E], F32, kind="ExternalOutput").ap()

    with tile.TileContext(nc) as tc:
        with (
            tc.tile_pool(name="consts", bufs=1) as cpool,
            tc.tile_pool(name="xp", bufs=1) as xpool,
            tc.tile_pool(name="qk", bufs=1) as qkpool,
            tc.tile_pool(name="vp2", bufs=2) as vpool,
            tc.tile_pool(name="vy", bufs=1) as vypool,
            tc.tile_pool(name="pt", bufs=6) as ptpool,
            tc.tile_pool(name="sm", bufs=2) as spool,
            tc.tile_pool(name="dn", bufs=1) as dnpool,
            tc.tile_pool(name="ob", bufs=3) as opool,
            tc.tile_pool(name="pso", bufs=2, space="PSUM") as pso,
            tc.tile_pool(name="psc", bufs=4, space="PSUM") as psc,
        ):
            # DMA order matters for the startup critical path: V-projection
            # only needs Wv + xT, so those go first; Wo/bias are not needed
            # until the output projection
            wq_sb = cpool.tile([P, ET, H * Dh], BF, tag="wq")
            wk_sb = cpool.tile([P, ET, H * Dh], BF, tag="wk")
            wv_sb = cpool.tile([P, ET, H * Dh], BF, tag="wv")
            wo_sb = cpool.tile([P, ET, E], BF, tag="wo")
            nc.sync.dma_start(wv_sb[:], wv.rearrange("(n p) c -> p n c", p=P))

            xT_tiles = {}
            v_tiles = {}

            def load_blocks(b):
                """xT load + V-projection for batch b as dense PE filler
                blocks (also usable as pending entries during the previous
                batch's last attention pairs)."""
                def ld(b=b):
                    xT_tiles[b] = xpool.tile([P, ET, T], BF, tag="xT",
                                             name=f"xT{b}")
                    nc.sync.dma_start(
                        xT_tiles[b][:],
                        xT[b].rearrange("(n p) c -> p n c", p=P))
                    v_tiles[b] = vpool.tile([P, ET, H, Dh + 1], BF, tag="v",
                                            name=f"v{b}")
                    nc.vector.memset(v_tiles[b][:, :, :, Dh], 1.0)
                blocks = [ld]
                for t in range(ET):
                    for n2 in range(2):
                        def vblk(t=t, n2=n2, b=b):
                            cs = slice(512 * n2, 512 * (n2 + 1))
                            vp = psc.tile([P, 512], F32, tag="pc",
                                          name=f"vp{b}_{t}_{n2}")
                            for i in range(ET):
                                nc.tensor.matmul(
                                    vp[:],
                                    lhsT=xT_tiles[b][:, i,
                                                     128 * t:128 * (t + 1)],
                                    rhs=wv_sb[:, i, cs],
                                    start=(i == 0), stop=(i == ET - 1),
                                )
                            nc.scalar.activation(
                                v_tiles[b][:, t, 8 * n2:8 * (n2 + 1), 0:Dh],
                                vp[:].rearrange("p (h d) -> p h d", d=Dh),
                                AF.Copy,
                            )
                        blocks.append(vblk)
                return blocks

            blocks0 = load_blocks(0)
            blocks0[0]()  # xT load right behind Wv
            nc.sync.dma_start(wq_sb[:], wq.rearrange("(n p) c -> p n c", p=P))
            nc.sync.dma_start(wk_sb[:], wk.rearrange("(n p) c -> p n c", p=P))
            mask_sb = cpool.tile([P, P], BF, tag="mask")
            nc.sync.dma_start(mask_sb[:], mask01)
            for blk in blocks0[1:]:
                blk()
            nc.sync.dma_start(wo_sb[:], wo.rearrange("(n p) c -> p n c", p=P))
            borep_sb = cpool.tile([P, E], F32, tag="bo")
            nc.sync.dma_start(borep_sb[:], borep)

            pending = []

            def drain(n):
                for _ in range(min(n, len(pending))):
                    pending.pop(0)()

            for b in range(BL):
                xT_sb = xT_tiles[b]
                v_sb = v_tiles[b]

                # ---- Q^T / K^T projections, emitted as closures so pair
                # pp's projection interleaves into pair pp-1's attention ----
                qT = qkpool.tile([P, HP, T], BF, tag="q", name=f"q{b}")
                kT = qkpool.tile([P, HP, T], BF, tag="k", name=f"k{b}")

                def proj_subblocks(pp, b=b, qT=qT, kT=kT, xT_sb=xT_sb):
                    blocks = []
                    for (lbl, w_sb, dst, scale) in (("q", wq_sb, qT, 0.125),
                                                    ("k", wk_sb, kT, 1.0)):
                        for n2 in range(2):
                            def blk(lbl=lbl, w_sb=w_sb, dst=dst, scale=scale,
                                    n2=n2, pp=pp, b=b):
                                cs = slice(512 * n2, 512 * (n2 + 1))
                                pj = psc.tile(
                                    [P, 512], F32, tag="pc",
                                    name=f"pj{b}_{pp}_{n2}_{lbl}")
                                for i in range(ET):
                                    nc.tensor.matmul(
                                        pj[:],
                                        lhsT=w_sb[:, i,
                                                  128 * pp:128 * (pp + 1)],
                                        rhs=xT_sb[:, i, cs],
                                        start=(i == 0), stop=(i == ET - 1),
                                    )
                                nc.scalar.activation(dst[:, pp, cs], pj[:],
                                                     AF.Copy, scale=scale)
                            blocks.append(blk)
                    return blocks

                for blk in proj_subblocks(0):
                    blk()

                # ---- attention: pairs of heads, drip-scheduled extras ----
                yT = vypool.tile([P, HP, T], BF, tag="y", name=f"y{b}")

                def yT_ap(hp):
                    return yT[:, hp, :]
                den = dnpool.tile([P, 4, T], F32, tag="den")
                nc.vector.memset(den[:], 1.0)

                def queue_normalize(g, b=b, yT_ap=yT_ap, den=den,
                                    half=None):
                    # half=0/1 reciprocates only partitions [0:64)/[64:128)
                    # of the slot (heads 4g..4g+1 / 4g+2..4g+3) so the last
                    # quad can normalize pair-by-pair instead of in one
                    # end-of-batch burst
                    p0, p1 = (0, P) if half is None else \
                        (64 * half, 64 * half + 64)
                    for c in range(8):
                        def recip_chunk(g=g, c=c, p0=p0, p1=p1):
                            nc.vector.reciprocal(
                                den[p0:p1, g, 128 * c:128 * (c + 1)],
                                den[p0:p1, g, 128 * c:128 * (c + 1)])
                        pending.append(recip_chunk)
                    if dbg and b == 0 and g == 0 and half is None:
                        pending.append(lambda: nc.sync.dma_start(
                            dbg_out["d_r1"], den[0:1, 0, :]))
                    heads = range(4 * g, 4 * g + 4) if half is None else \
                        range(4 * g + 2 * half, 4 * g + 2 * half + 2)
                    for h in heads:
                        holder = {}

                        def stage(h=h, b=b, holder=None):
                            hp, po = h // 2, Dh * (h % 2)
                            pb = 32 * (h % 4)
                            r1 = spool.tile([1, T], BF, tag="r1",
                                            name=f"r1_{b}_{h}")
                            nc.scalar.activation(
                                r1[:], den[pb:pb + 1, h // 4, :], AF.Copy)
                            rb = spool.tile([P, T], BF, tag="rb",
                                            name=f"rb_{b}_{h}")
                            nc.gpsimd.partition_broadcast(rb[:], r1[:])
                            if dbg and b == 0 and h == 1:
                                nc.sync.dma_start(dbg_out["d_rb"],
                                                  rb[0:Dh, :])
                            holder['rb'] = rb

                        def mul_step(h=h, holder=holder):
                            hp, po = h // 2, Dh * (h % 2)
                            ap = yT_ap(hp)
                            nc.vector.tensor_mul(
                                ap[po:po + Dh, :], ap[po:po + Dh, :],
                                holder['rb'][po:po + Dh, :])

                        pending.append(
                            lambda h=h, holder=holder: stage(h, b, holder))
                        pending.append(mul_step)

                for hp in range(HP):
                    if hp + 1 < HP:
                        # front of the queue: pair hp+1's projection must
                        # finish within this pair's attention
                        pending[0:0] = proj_subblocks(hp + 1)
                    if hp == 6 and b + 1 < BL:
                        # next batch's x load + V projection: dense PE
                        # filler for the last two pairs (which have no
                        # projection blocks of their own)
                        pending.extend(load_blocks(b + 1))
                    ops = [pso.tile([P, 1024], F32, tag="op",
                                    name=f"op{b}_{hp}_{s}") for s in range(2)]
                    for i in range(ET):
                        pts = []
                        for sub in (0, 1):
                            po = Dh * sub
                            pt = ptpool.tile([P, 1024], BF, tag="pt",
                                             name=f"pt{b}_{hp}_{i}_{sub}")
                            for (a0, a1) in _pieces(i):
                                sp_ = psc.tile([P, 512], F32, tag="pc",
                                               name=f"sp{b}_{hp}_{i}_{sub}_{a0}")
                                w = a1 - a0
                                nc.tensor.matmul(
                                    sp_[:, 0:w],
                                    lhsT=kT[po:po + Dh, hp,
                                            128 * i:128 * (i + 1)],
                                    rhs=qT[po:po + Dh, hp, a0:a1],
                                    start=True, stop=True,
                                )
                                nc.scalar.activation(pt[:, a0:a1],
                                                     sp_[:, 0:w], AF.Exp)
                            ds_ = slice(128 * i, 128 * (i + 1))
                            nc.vector.tensor_mul(pt[:, ds_], pt[:, ds_],
                                                 mask_sb[:])
                            pts.append(pt)
                            if dbg and b == 0 and hp == 0 and sub == 0:
                                nc.sync.dma_start(dbg_out["d_pt"][i], pt[:])
                        for sub in (0, 1):
                            h = 2 * hp + sub
                            for (a0, a1) in _pieces(i):
                                nc.tensor.matmul(
                                    ops[sub][0:Dh + 1, a0:a1],
                                    lhsT=v_sb[:, i, h, :],
                                    rhs=pts[sub][:, a0:a1],
                                    start=(i == 0), stop=(i == ET - 1),
                                    skip_group_check=True,
                                )
                            if i == ET - 1:
                                po = Dh * sub
                                nc.scalar.activation(
                                    yT_ap(hp)[po:po + Dh, :],
                                    ops[sub][0:Dh, :], AF.Copy)
                                pb = 32 * (h % 4)
                                nc.vector.tensor_copy(
                                    den[pb:pb + 1, h // 4, :],
                                    ops[sub][Dh:Dh + 1, :])
                                if dbg and b == 0 and h == 0:
                                    opc = dnpool.tile([P, T], F32,
                                                      tag="dbg_op")
                                    nc.vector.tensor_copy(opc[:],
                                                          ops[sub][:])
                                    nc.sync.dma_start(dbg_out["d_op"],
                                                      opc[:])
                            drain(1)
                    if hp == HP - 2:
                        queue_normalize(3, half=0)
                    elif hp == HP - 1:
                        queue_normalize(3, half=1)
                    elif hp % 2 == 1:
                        queue_normalize(hp // 2)

                # emission order IS dependency order under Tile's tracer:
                # all normalize muls must be emitted before out-proj reads yT
                drain(len(pending))
                if dbg and b == 0:
                    nc.sync.dma_start(dbg_out["d_qT"], qT[:])
                    nc.sync.dma_start(dbg_out["d_kT"], kT[:])
                    nc.sync.dma_start(dbg_out["d_v"], v_sb[:])
                    nc.sync.dma_start(dbg_out["d_yT"], yT[:])

                # ---- output projection + bias ----
                # for non-final batches the half-blocks are deferred into the
                # NEXT batch's attention stream (dense PE filler that
                # overlaps what used to be a serial phase)
                def outproj_blocks(b=b, yT=yT):
                    blocks = []
                    for t in range(ET):
                        for n2 in range(2):
                            def oblk(t=t, n2=n2, b=b, yT=yT):
                                cs = slice(512 * n2, 512 * (n2 + 1))
                                o2 = psc.tile([P, 512], F32, tag="pc",
                                              name=f"o2_{b}_{t}_{n2}")
                                for j in range(ET):
                                    nc.tensor.matmul(
                                        o2[:],
                                        lhsT=yT[:, j,
                                                128 * t:128 * (t + 1)],
                                        rhs=wo_sb[:, j, cs],
                                        start=(j == 0), stop=(j == ET - 1),
                                    )
                                ob = opool.tile([P, 512], F32, tag="ob",
                                                name=f"ob{b}_{t}_{n2}")
                                nc.vector.tensor_add(ob[:], o2[:],
                                                     borep_sb[:, cs])
                                nc.sync.dma_start(
                                    out[b, 128 * t:128 * (t + 1), cs],
                                    ob[:])
                            blocks.append(oblk)
                    return blocks

                # b's normalize must be fully emitted before anything of
                # batch b+1 (den/yT pool slots are reused)
                drain(len(pending))
                blocks = outproj_blocks()
                if b + 1 < BL:
                    # defer half into the next batch's attention stream;
                    # they must all drain before b+1's first yT write
                    # (pair-0 end), which bounds the deferral to ~12 entries
                    for blk in blocks[:8]:
                        blk()
                    pending.extend(blocks[8:])
                else:
                    for blk in blocks:
                        blk()
            drain(len(pending))

    nc.compile()
    return nc


def _get_nc():
    if "nc" not in _CACHE:
        _CACHE["nc"] = _build()
    return _CACHE["nc"]


def _prep_in_maps(x, Wq, Wk, Wv, Wo, bo):
    bf16 = ml_dtypes.bfloat16
    # [B,T,E] -> [B,E,T] transposed activations
    xT = np.ascontiguousarray(np.asarray(x).transpose(0, 2, 1)).astype(bf16)
    # [H,E,Dh] -> [E, H*Dh] (heads side by side so a 128-col slice = 2 heads)
    wq_pk = np.ascontiguousarray(
        np.asarray(Wq).transpose(1, 0, 2).reshape(E, H * Dh)).astype(bf16)
    wk_pk = np.ascontiguousarray(
        np.asarray(Wk).transpose(1, 0, 2).reshape(E, H * Dh)).astype(bf16)
    wv_pk = np.ascontiguousarray(
        np.asarray(Wv).transpose(1, 0, 2).reshape(E, H * Dh)).astype(bf16)
    wo_b = np.ascontiguousarray(np.asarray(Wo)).astype(bf16)
    borep = np.ascontiguousarray(
        np.broadcast_to(np.asarray(bo, np.float32), (P, E)))
    ii, jj = np.mgrid[0:P, 0:P]
    mask01 = (jj >= ii).astype(bf16)  # S^T[tk,tq]: keep tq >= tk

    in_maps = []
    for c in range(NCORES):
        in_maps.append({
            "xT": xT[BL * c:BL * (c + 1)],
            "wq": wq_pk, "wk": wk_pk, "wv": wv_pk, "wo": wo_b,
            "borep": borep, "mask01": mask01,
        })
    return in_maps


def run(inputs, trace=False):
    """Returns (full_output [B,T,E] fp32, BassKernelResults)."""
    nc = _get_nc()
    in_maps = _prep_in_maps(**inputs)
    res = run_bass_kernel_spmd(nc, in_maps, core_ids=list(range(NCORES)),
                               trace=trace)
    out = np.concatenate([res.results[c]["out"] for c in range(NCORES)],
                         axis=0)
    return out, res


def kernel(x, Wq, Wk, Wv, Wo, bo):
    out, _ = run(dict(x=x, Wq=Wq, Wk=Wk, Wv=Wv, Wo=Wo, bo=bo))
    return out
